# revision 22
# baseline (speedup 1.0000x reference)
"""Trainium2 Bass kernel for nn_Concat_Linear (feat [65536,2,768] -> out [65536,9]).

Data-parallel across 8 NeuronCores (8192 rows each). Per core:

  - feat is DMA'd in its NATURAL row-major layout through the gpsimd SWDGE
    queue, casting fp32 -> fp16 in the DMA datapath. Partition p holds 4
    consecutive rows (p*4+n), i.e. ONE contiguous 24KB descriptor per
    partition per 512-row group, so both the HBM read and the Q7 descriptor
    generation are cheap (the original 192B-descriptor scrambled load ran at
    ~50% HBM rate; this runs at line rate, ~127us of DMA-engine time).
  - feature-on-partition tiles tT[p, m, n*128+r] = feat[row 4r+n, m*128+p]
    are produced by PE is_transpose matmuls (fp16 128x128: warm steady-state
    ~56ns each since LDWEIGHTS of the next overlaps the current stream),
    drained from PSUM by wide DVE/ACT copies.
  - the projection is 12 K=128 accumulating fp16 matmuls per 512-row group,
    yielding Y = [this | last] with this at partitions 0:9, last at 32:41.
  - the trilinear form, LayerNorm and final linear run in feature-on-partition
    orientation via small PE matmuls + DVE/ACT elementwise ops; ln_w/ln_b are
    folded host-side into the final weights/bias. rstd comes from
    ACT Sqrt(var+eps) (single table, no ACT_TABLE_LOAD churn) -> PE broadcast
    to 9 partitions -> DVE reciprocal_approx_fast (the exact InstReciprocal
    costs 3.3us per call). DVE reads bil/g/rb straight from PSUM, skipping
    two copy stages.
  - the whole thing is software-pipelined 3 deep: front(g) // epiA(g-1) //
    epiB(g-2), with the per-engine emission order hand-interleaved so the
    in-order PE queue never waits long on the DVE/ACT latency chain and the
    PE stays in the warm (2.4 GHz) HAM state.
  - outputs are PE-transposed back to row-major (4 rows per partition, 144B
    contiguous store runs) and stored once per 512 rows.
"""

import sys
import types

import numpy as np

B_FULL = 65536
N_CORES = 8
B_CORE = B_FULL // N_CORES
D = 1536  # 2 * 768
NG = 512  # rows per group
NCH = NG // 128  # 128-row chunks per group (4)
C = D // 128     # 128-feature chunks (12)
LN_EPS = 1e-5


def _ensure_axon_hooks():
    """Register the NTFF profile hook if the image's antenv lacks axon_hooks.

    Without this, trace=True degrades to no profiling (runs still work)."""
    try:
        import antenv  # noqa: F401
        from antenv import axon_hooks  # noqa: F401
        return
    except ImportError:
        pass
    try:
        import antenv
        mod = types.ModuleType("antenv.axon_hooks")
        mod._hook = None
        mod.set_axon_ntff_profile_hook = lambda h: setattr(mod, "_hook", h)
        mod.get_axon_ntff_profile_hook = lambda: mod._hook
        sys.modules["antenv.axon_hooks"] = mod
        antenv.axon_hooks = mod
        from trn_agent_boot.trn_boot import _ntff_profile_via_ctypes
        mod.set_axon_ntff_profile_hook(
            _ntff_profile_via_ctypes("/opt/axon/libaxon_pjrt.so")
        )
    except Exception:
        pass


def make_consts(W_int, W_stim, trans, ln_w, ln_b, W_out, b_out):
    """Host-side constant tensors."""
    W_int = np.asarray(W_int, np.float32)
    W_stim = np.asarray(W_stim, np.float32)
    trans = np.asarray(trans, np.float32)
    ln_w = np.asarray(ln_w, np.float32)
    ln_b = np.asarray(ln_b, np.float32)
    W_out = np.asarray(W_out, np.float32)
    b_out = np.asarray(b_out, np.float32)

    # Projection weights: Y[:, 0:9] = this = feat[:,1,:] @ W_stim.T
    #                     Y[:, 32:41] = last = feat[:,0,:] @ W_int.T
    W_cat = np.zeros((D, 41), np.float32)
    W_cat[768:1536, 0:9] = W_stim.T
    W_cat[0:768, 32:41] = W_int.T
    # transpose layout: feature m*128+p sits on partition p, slot m
    wsc = np.ascontiguousarray(
        W_cat.reshape(C, 128, 41).transpose(1, 0, 2)).astype(np.float16)

    # trans matrix for G[a*9+k, b] = sum_j trans[a,j,k] * last[j, b]
    # rows live at partitions 32:41 to match last's position in Y.
    tm = np.zeros((41, 81), np.float32)
    for a in range(9):
        for j in range(9):
            for k in range(9):
                tm[32 + j, a * 9 + k] = trans[a, j, k]

    # thisbc[a*9+k, b] = this[a, b]
    e9 = np.zeros((9, 81), np.float32)
    for a in range(9):
        e9[a, a * 9:(a + 1) * 9] = 1.0

    # bil_centered[k', b] = sum_a M[a*9+k', b] - (1/9) sum_rows M[row, b]
    rp = np.full((81, 9), -1.0 / 9.0, np.float32)
    for a in range(9):
        for k in range(9):
            rp[a * 9 + k, k] += 1.0

    o99 = np.full((9, 1), 1.0 / 9.0, np.float32)   # mean-of-squares reducer
    o19 = np.ones((1, 9), np.float32)              # std partition-broadcast

    # Final linear with ln_w/ln_b folded in:
    # out = W_out[:, :9] @ this + (W_out[:, 9:] * ln_w) @ (bil_c * rstd) + b'
    l1 = np.ascontiguousarray(W_out[:, 0:9].T)
    l2 = np.ascontiguousarray((W_out[:, 9:18] * ln_w[None, :]).T)
    bout = (b_out + W_out[:, 9:18] @ ln_b).reshape(9, 1).astype(np.float32)

    i9 = np.eye(9, dtype=np.float32)
    i128 = np.eye(128, dtype=np.float16)

    return {
        "wsc": wsc, "tm": tm, "e9": e9, "rp": rp, "o99": o99, "o19": o19,
        "l1": l1, "l2": l2, "bout": bout, "i9": i9, "i128": i128,
        "eps": np.full((1, 1), LN_EPS, np.float32),
    }


def build_program(b_core=B_CORE, num_devices=N_CORES):
    import concourse.bass as bass  # noqa: F401
    import concourse.tile as tile
    from concourse import bacc, mybir

    f32 = mybir.dt.float32
    f32r = mybir.dt.float32r
    f16 = mybir.dt.float16
    nc = bacc.Bacc("TRN2", target_bir_lowering=False, debug=False,
                   num_devices=num_devices)

    feat_d = nc.dram_tensor("feat", [b_core, D], f32, kind="ExternalInput")
    out_d = nc.dram_tensor("out", [b_core, 9], f32, kind="ExternalOutput")
    cshapes = {
        "wsc": [128, C, 41], "tm": [41, 81], "e9": [9, 81], "rp": [81, 9],
        "o99": [9, 1], "o19": [1, 9], "l1": [9, 9], "l2": [9, 9],
        "bout": [9, 1], "i9": [9, 9], "i128": [128, 128], "eps": [1, 1],
    }
    f32r_keys = {"tm", "e9", "rp", "o99", "o19", "l1", "l2"}

    def cdt(k):
        if k in ("wsc", "i128"):
            return f16
        return f32r if k in f32r_keys else f32

    cd = {k: nc.dram_tensor(k, v, cdt(k), kind="ExternalInput")
          for k, v in cshapes.items()}

    ngrp = b_core // NG
    st = {}  # per-group pipeline state: g -> dict of tiles
    with tile.TileContext(nc) as tc:
        with tc.tile_pool(name="consts", bufs=1) as cp, \
             tc.tile_pool(name="natp", bufs=10) as natp, \
             tc.tile_pool(name="ttp", bufs=2) as ttp, \
             tc.tile_pool(name="ysb", bufs=3) as ysbp, \
             tc.tile_pool(name="episb", bufs=3) as esbp, \
             tc.tile_pool(name="outsb", bufs=2) as outp, \
             tc.tile_pool(name="tps", bufs=3, space="PSUM") as tpp, \
             tc.tile_pool(name="yps", bufs=2, space="PSUM") as yp, \
             tc.tile_pool(name="epips", bufs=3, space="PSUM") as epp:

            cs = {k: cp.tile(v, cdt(k), tag=k, name=k)
                  for k, v in cshapes.items()}
            for k in cshapes:
                nc.scalar.dma_start(cs[k][:], cd[k].ap())

            def emit_load(g):
                # partition p holds rows g*512 + p*4 + n; one 6KB run per
                # partition per sub-load. 4 sub-loads per group so each
                # transpose chunk waits only on its own 768KB slice (a single
                # whole-group load made the first consumer wait on every
                # outstanding load and shifted the whole PE pipeline ~20us).
                nats = []
                for n in range(NCH):
                    nat = natp.tile([128, D], f16, tag="nat",
                                    name=f"nat{g}_{n}")
                    nc.gpsimd.dma_start(
                        nat[:],
                        feat_d.ap()[g * NG + n:(g + 1) * NG:NCH, :])
                    nats.append(nat)
                tT = ttp.tile([128, C, NG], f16, tag="tT", name=f"tT{g}")
                st[g] = {"nat": nats, "tT": tT}

            def emit_T(g, n):
                # PE-transpose row-chunk n: tT[p, c, n*128+r] = feat fp16
                nat, tT = st[g]["nat"][n], st[g]["tT"]
                for t in range(3):
                    tp = tpp.tile([128, 4, 128], f16, tag="tp",
                                  name=f"tp{g}_{n}_{t}")
                    for q in range(4):
                        c = t * 4 + q
                        nc.tensor.matmul(
                            tp[:, q, :],
                            nat[:, c * 128:(c + 1) * 128],
                            cs["i128"][:],
                            is_transpose=True,
                            start=(q == 0), stop=(q == 3))
                    dst = tT[:, t * 4:(t + 1) * 4, n * 128:(n + 1) * 128]
                    src = tp[:].rearrange("p a b -> p (a b)")
                    if t == 1 or (t == 2 and n % 2 == 1):
                        nc.scalar.copy(dst, src)
                    else:
                        nc.vector.tensor_copy(dst, src)

            def emit_P(g, half):
                tT = st[g]["tT"]
                if half == 0:
                    y_ps = yp.tile([41, NG], f32, tag="y", name=f"y{g}")
                    st[g]["y_ps"] = y_ps
                y_ps = st[g]["y_ps"]
                for m in range(half * 6, half * 6 + 6):
                    nc.tensor.matmul(
                        y_ps[:], cs["wsc"][:, m, :], tT[:, m, :],
                        start=(m == 0), stop=(m == C - 1))

            def emit_ycopy(g):
                y_sb = ysbp.tile([41, NG], f32r, tag="y_sb", name=f"y_sb{g}")
                nc.scalar.copy(y_sb[:], st[g]["y_ps"][:])
                st[g]["y_sb"] = y_sb

            def emit_A1(h):
                s = st[h]
                y_sb = s["y_sb"]
                s["g_ps"] = epp.tile([81, NG], f32, tag="ep", name=f"g{h}")
                nc.tensor.matmul(s["g_ps"][:], cs["tm"][32:41, :],
                                 y_sb[32:41, :], tile_position=(32, 0))
                s["tb_ps"] = epp.tile([81, NG], f32, tag="ep", name=f"tb{h}")
                nc.tensor.matmul(s["tb_ps"][:], cs["e9"][:], y_sb[0:9, :])
                s["tb_sb"] = esbp.tile([81, NG], f32, tag="tbs", name=f"tbs{h}")
                nc.scalar.copy(s["tb_sb"][:], s["tb_ps"][:])
                s["m_sb"] = esbp.tile([81, NG], f32r, tag="m", name=f"m{h}")
                nc.vector.tensor_mul(s["m_sb"][:], s["g_ps"][:], s["tb_sb"][:])

            def emit_A2(h):
                s = st[h]
                s["bil_ps"] = epp.tile([9, NG], f32, tag="ep", name=f"bil{h}")
                nc.tensor.matmul(s["bil_ps"][:], cs["rp"][:], s["m_sb"][:])
                s["bil_sb"] = esbp.tile([9, NG], f32, tag="bil",
                                        name=f"bils{h}")
                nc.scalar.copy(s["bil_sb"][:], s["bil_ps"][:])
                s["sq_sb"] = esbp.tile([9, NG], f32r, tag="sq", name=f"sq{h}")
                nc.vector.tensor_mul(s["sq_sb"][:], s["bil_sb"][:],
                                     s["bil_sb"][:])

            def emit_A3(h):
                s = st[h]
                s["var_ps"] = epp.tile([1, NG], f32, tag="ep", name=f"var{h}")
                nc.tensor.matmul(s["var_ps"][:], cs["o99"][:], s["sq_sb"][:])
                s["std_sb"] = esbp.tile([1, NG], f32r, tag="std",
                                        name=f"std{h}")
                nc.scalar.activation(s["std_sb"][:], s["var_ps"][:],
                                     mybir.ActivationFunctionType.Sqrt,
                                     bias=cs["eps"][:, 0:1])

            def emit_A4(h):
                s = st[h]
                s["rb_ps"] = epp.tile([9, NG], f32, tag="ep", name=f"rb{h}")
                nc.tensor.matmul(s["rb_ps"][:], cs["o19"][:], s["std_sb"][:])
                s["rstd_sb"] = esbp.tile([9, NG], f32, tag="rstd",
                                         name=f"rstd{h}")
                nc.vector.reciprocal_approx_fast(s["rstd_sb"][:],
                                                 s["rb_ps"][:])
                s["ln_sb"] = esbp.tile([9, NG], f32r, tag="ln", name=f"ln{h}")
                nc.vector.tensor_mul(s["ln_sb"][:], s["bil_sb"][:],
                                     s["rstd_sb"][:])

            def emit_B1(h):
                s = st[h]
                s["o_ps"] = epp.tile([9, NG], f32, tag="ep", name=f"o{h}")
                nc.tensor.matmul(s["o_ps"][:], cs["l2"][:], s["ln_sb"][:],
                                 start=True, stop=False)
                nc.tensor.matmul(s["o_ps"][:], cs["l1"][:],
                                 s["y_sb"][0:9, :], start=False, stop=True)
                s["osb"] = esbp.tile([9, NG], f32, tag="osb", name=f"osb{h}")
                nc.vector.tensor_scalar_add(s["osb"][:], s["o_ps"][:],
                                            cs["bout"][:, 0:1])

            def emit_B2(h):
                s = st[h]
                # chunk s covers columns s*128.. = rows 4r+s, so partition r
                # holds rows 4r..4r+3 (144B contiguous store runs)
                ot_ps = yp.tile([128, NCH, 9], f32, tag="y", name=f"ot{h}")
                for sx in range(NCH):
                    nc.tensor.matmul(
                        ot_ps[:, sx, :],
                        s["osb"][:, sx * 128:(sx + 1) * 128],
                        cs["i9"][:],
                        is_transpose=True,
                        start=(sx == 0), stop=(sx == NCH - 1))
                out_sb = outp.tile([128, NCH, 9], f32, tag="out_sb",
                                   name=f"outsb{h}")
                nc.vector.tensor_copy(out_sb[:], ot_ps[:])
                nc.scalar.dma_start(
                    out_d.ap()[h * NG:(h + 1) * NG, :]
                    .rearrange("(p s) k -> p s k", s=NCH),
                    out_sb[:])
                del st[h]

            # 3-deep software pipeline; PE emission order hand-interleaved
            for g in range(ngrp + 2):
                h1, h2 = g - 1, g - 2
                if g < ngrp:
                    emit_load(g)
                    emit_T(g, 0)
                    emit_T(g, 1)
                if 0 <= h1 < ngrp:
                    emit_A1(h1)
                if 0 <= h2 < ngrp:
                    emit_B1(h2)
                if g < ngrp:
                    emit_T(g, 2)
                if 0 <= h1 < ngrp:
                    emit_A2(h1)
                if g < ngrp:
                    emit_T(g, 3)
                if 0 <= h2 < ngrp:
                    emit_B2(h2)
                if g < ngrp:
                    emit_P(g, 0)
                if 0 <= h1 < ngrp:
                    emit_A3(h1)
                if g < ngrp:
                    emit_P(g, 1)
                if 0 <= h1 < ngrp:
                    emit_A4(h1)
                if g < ngrp:
                    emit_ycopy(g)
    nc.compile()
    return nc


_PROGRAM = None


def _get_program():
    global _PROGRAM
    if _PROGRAM is None:
        _PROGRAM = build_program()
    return _PROGRAM


def kernel(feat, W_int, W_stim, trans, ln_w, ln_b, W_out, b_out,
           trace=False, trace_kwargs=None):
    _ensure_axon_hooks()
    from concourse.bass_utils import run_bass_kernel_spmd

    feat = np.asarray(feat, np.float32)
    feat2 = feat.reshape(B_FULL, D)
    consts = make_consts(W_int, W_stim, trans, ln_w, ln_b, W_out, b_out)
    nc = _get_program()
    in_maps = []
    for c in range(N_CORES):
        m = {"feat": np.ascontiguousarray(feat2[c * B_CORE:(c + 1) * B_CORE])}
        m.update(consts)
        in_maps.append(m)
    res = run_bass_kernel_spmd(nc, in_maps, list(range(N_CORES)), trace=trace)
    out = np.concatenate([res.results[c]["out"] for c in range(N_CORES)], axis=0)
    kernel.last_results = res
    return np.ascontiguousarray(out, dtype=np.float32)


# revision 23
# speedup vs baseline: 1.0684x; 1.0684x over previous
"""Trainium2 Bass kernel for nn_Concat_Linear (feat [65536,2,768] -> out [65536,9]).

Data-parallel across 8 NeuronCores (8192 rows each). Per core:

  - feat is DMA'd in its NATURAL row-major layout through the gpsimd SWDGE
    queue, casting fp32 -> fp16 in the DMA datapath. Partition p holds 4
    consecutive rows (p*4+n), i.e. ONE contiguous 24KB descriptor per
    partition per 512-row group, so both the HBM read and the Q7 descriptor
    generation are cheap (the original 192B-descriptor scrambled load ran at
    ~50% HBM rate; this runs at line rate, ~127us of DMA-engine time).
  - feature-on-partition tiles tT[p, m, n*128+r] = feat[row 4r+n, m*128+p]
    are produced by PE is_transpose matmuls (fp16 128x128: warm steady-state
    ~56ns each since LDWEIGHTS of the next overlaps the current stream),
    drained from PSUM by wide DVE/ACT copies.
  - the projection is 12 K=128 accumulating fp16 matmuls per 512-row group,
    yielding Y = [this | last] with this at partitions 0:9, last at 32:41.
  - the trilinear form, LayerNorm and final linear run in feature-on-partition
    orientation via small PE matmuls + DVE/ACT elementwise ops; ln_w/ln_b are
    folded host-side into the final weights/bias. rstd comes from
    ACT Sqrt(var+eps) (single table, no ACT_TABLE_LOAD churn) -> PE broadcast
    to 9 partitions -> DVE reciprocal_approx_fast (the exact InstReciprocal
    costs 3.3us per call). DVE reads bil/g/rb straight from PSUM, skipping
    two copy stages.
  - the whole thing is software-pipelined 3 deep: front(g) // epiA(g-1) //
    epiB(g-2), with the per-engine emission order hand-interleaved so the
    in-order PE queue never waits long on the DVE/ACT latency chain and the
    PE stays in the warm (2.4 GHz) HAM state.
  - outputs are PE-transposed back to row-major (4 rows per partition, 144B
    contiguous store runs) and stored once per 512 rows.
"""

import sys
import types

import numpy as np

B_FULL = 65536
N_CORES = 8
B_CORE = B_FULL // N_CORES
D = 1536  # 2 * 768
NG = 512  # rows per group
NCH = NG // 128  # 128-row chunks per group (4)
C = D // 128     # 128-feature chunks (12)
LN_EPS = 1e-5


def _ensure_axon_hooks():
    """Register the NTFF profile hook if the image's antenv lacks axon_hooks.

    Without this, trace=True degrades to no profiling (runs still work)."""
    try:
        import antenv  # noqa: F401
        from antenv import axon_hooks  # noqa: F401
        return
    except ImportError:
        pass
    try:
        import antenv
        mod = types.ModuleType("antenv.axon_hooks")
        mod._hook = None
        mod.set_axon_ntff_profile_hook = lambda h: setattr(mod, "_hook", h)
        mod.get_axon_ntff_profile_hook = lambda: mod._hook
        sys.modules["antenv.axon_hooks"] = mod
        antenv.axon_hooks = mod
        from trn_agent_boot.trn_boot import _ntff_profile_via_ctypes
        mod.set_axon_ntff_profile_hook(
            _ntff_profile_via_ctypes("/opt/axon/libaxon_pjrt.so")
        )
    except Exception:
        pass


def make_consts(W_int, W_stim, trans, ln_w, ln_b, W_out, b_out):
    """Host-side constant tensors."""
    W_int = np.asarray(W_int, np.float32)
    W_stim = np.asarray(W_stim, np.float32)
    trans = np.asarray(trans, np.float32)
    ln_w = np.asarray(ln_w, np.float32)
    ln_b = np.asarray(ln_b, np.float32)
    W_out = np.asarray(W_out, np.float32)
    b_out = np.asarray(b_out, np.float32)

    # Projection weights: Y[:, 0:9] = this = feat[:,1,:] @ W_stim.T
    #                     Y[:, 32:41] = last = feat[:,0,:] @ W_int.T
    W_cat = np.zeros((D, 41), np.float32)
    W_cat[768:1536, 0:9] = W_stim.T
    W_cat[0:768, 32:41] = W_int.T
    # transpose layout: feature m*128+p sits on partition p, slot m
    wsc = np.ascontiguousarray(
        W_cat.reshape(C, 128, 41).transpose(1, 0, 2)).astype(np.float16)

    # trans matrix for G[a*9+k, b] = sum_j trans[a,j,k] * last[j, b]
    # rows live at partitions 32:41 to match last's position in Y.
    tm = np.zeros((41, 81), np.float32)
    for a in range(9):
        for j in range(9):
            for k in range(9):
                tm[32 + j, a * 9 + k] = trans[a, j, k]

    # thisbc[a*9+k, b] = this[a, b]
    e9 = np.zeros((9, 81), np.float32)
    for a in range(9):
        e9[a, a * 9:(a + 1) * 9] = 1.0

    # bil_centered[k', b] = sum_a M[a*9+k', b] - (1/9) sum_rows M[row, b]
    rp = np.full((81, 9), -1.0 / 9.0, np.float32)
    for a in range(9):
        for k in range(9):
            rp[a * 9 + k, k] += 1.0

    o99 = np.full((9, 1), 1.0 / 9.0, np.float32)   # mean-of-squares reducer
    o19 = np.ones((1, 9), np.float32)              # std partition-broadcast

    # Final linear with ln_w/ln_b folded in:
    # out = W_out[:, :9] @ this + (W_out[:, 9:] * ln_w) @ (bil_c * rstd) + b'
    l1 = np.ascontiguousarray(W_out[:, 0:9].T)
    l2 = np.ascontiguousarray((W_out[:, 9:18] * ln_w[None, :]).T)
    bout = (b_out + W_out[:, 9:18] @ ln_b).reshape(9, 1).astype(np.float32)

    i9 = np.eye(9, dtype=np.float32)
    i128 = np.eye(128, dtype=np.float16)

    return {
        "wsc": wsc, "tm": tm, "e9": e9, "rp": rp, "o99": o99, "o19": o19,
        "l1": l1, "l2": l2, "bout": bout, "i9": i9, "i128": i128,
        "eps": np.full((1, 1), LN_EPS, np.float32),
    }


def build_program(b_core=B_CORE, num_devices=N_CORES):
    import concourse.bass as bass  # noqa: F401
    import concourse.tile as tile
    from concourse import bacc, mybir

    f32 = mybir.dt.float32
    f32r = mybir.dt.float32r
    f16 = mybir.dt.float16
    nc = bacc.Bacc("TRN2", target_bir_lowering=False, debug=False,
                   num_devices=num_devices)

    feat_d = nc.dram_tensor("feat", [b_core, D], f32, kind="ExternalInput")
    out_d = nc.dram_tensor("out", [b_core, 9], f32, kind="ExternalOutput")
    cshapes = {
        "wsc": [128, C, 41], "tm": [41, 81], "e9": [9, 81], "rp": [81, 9],
        "o99": [9, 1], "o19": [1, 9], "l1": [9, 9], "l2": [9, 9],
        "bout": [9, 1], "i9": [9, 9], "i128": [128, 128], "eps": [1, 1],
    }
    f32r_keys = {"tm", "e9", "rp", "o99", "o19", "l1", "l2"}

    def cdt(k):
        if k in ("wsc", "i128"):
            return f16
        return f32r if k in f32r_keys else f32

    cd = {k: nc.dram_tensor(k, v, cdt(k), kind="ExternalInput")
          for k, v in cshapes.items()}

    ngrp = b_core // NG
    st = {}  # per-group pipeline state: g -> dict of tiles
    with tile.TileContext(nc) as tc:
        with tc.tile_pool(name="consts", bufs=1) as cp, \
             tc.tile_pool(name="natp", bufs=8) as natp, \
             tc.tile_pool(name="ttp", bufs=2) as ttp, \
             tc.tile_pool(name="ysb", bufs=3) as ysbp, \
             tc.tile_pool(name="episb", bufs=3) as esbp, \
             tc.tile_pool(name="outsb", bufs=2) as outp, \
             tc.tile_pool(name="tps", bufs=3, space="PSUM") as tpp, \
             tc.tile_pool(name="yps", bufs=2, space="PSUM") as yp, \
             tc.tile_pool(name="epips", bufs=3, space="PSUM") as epp:

            cs = {k: cp.tile(v, cdt(k), tag=k, name=k)
                  for k, v in cshapes.items()}
            for k in cshapes:
                nc.scalar.dma_start(cs[k][:], cd[k].ap())

            def emit_load(g):
                # partition p holds rows g*512 + p*4 + n; one 6KB run per
                # partition per sub-load. 4 sub-loads per group so each
                # transpose chunk waits only on its own 768KB slice (a single
                # whole-group load made the first consumer wait on every
                # outstanding load and shifted the whole PE pipeline ~20us).
                nats = []
                for n in range(NCH):
                    nat = natp.tile([128, D], f16, tag="nat",
                                    name=f"nat{g}_{n}")
                    nc.gpsimd.dma_start(
                        nat[:],
                        feat_d.ap()[g * NG + n:(g + 1) * NG:NCH, :])
                    nats.append(nat)
                tT = ttp.tile([128, C, NG], f16, tag="tT", name=f"tT{g}")
                st[g] = {"nat": nats, "tT": tT}

            def emit_T(g, n):
                # PE-transpose row-chunk n: tT[p, c, n*128+r] = feat fp16
                nat, tT = st[g]["nat"][n], st[g]["tT"]
                for t in range(3):
                    tp = tpp.tile([128, 4, 128], f16, tag="tp",
                                  name=f"tp{g}_{n}_{t}")
                    for q in range(4):
                        c = t * 4 + q
                        nc.tensor.matmul(
                            tp[:, q, :],
                            nat[:, c * 128:(c + 1) * 128],
                            cs["i128"][:],
                            is_transpose=True,
                            start=(q == 0), stop=(q == 3))
                    dst = tT[:, t * 4:(t + 1) * 4, n * 128:(n + 1) * 128]
                    src = tp[:].rearrange("p a b -> p (a b)")
                    if t == 1:
                        nc.scalar.copy(dst, src)
                    else:
                        nc.vector.tensor_copy(dst, src)

            def emit_P(g, half):
                tT = st[g]["tT"]
                if half == 0:
                    y_ps = yp.tile([41, NG], f32, tag="y", name=f"y{g}")
                    st[g]["y_ps"] = y_ps
                y_ps = st[g]["y_ps"]
                for m in range(half * 6, half * 6 + 6):
                    nc.tensor.matmul(
                        y_ps[:], cs["wsc"][:, m, :], tT[:, m, :],
                        start=(m == 0), stop=(m == C - 1))

            def emit_ycopy(g):
                y_sb = ysbp.tile([41, NG], f32r, tag="y_sb", name=f"y_sb{g}")
                nc.scalar.copy(y_sb[:], st[g]["y_ps"][:])
                st[g]["y_sb"] = y_sb

            def emit_A1(h):
                s = st[h]
                y_sb = s["y_sb"]
                s["g_ps"] = epp.tile([81, NG], f32, tag="ep", name=f"g{h}")
                nc.tensor.matmul(s["g_ps"][:], cs["tm"][32:41, :],
                                 y_sb[32:41, :], tile_position=(32, 0))
                s["tb_ps"] = epp.tile([81, NG], f32, tag="ep", name=f"tb{h}")
                nc.tensor.matmul(s["tb_ps"][:], cs["e9"][:], y_sb[0:9, :])
                s["tb_sb"] = esbp.tile([81, NG], f32, tag="tbs", name=f"tbs{h}")
                nc.scalar.copy(s["tb_sb"][:], s["tb_ps"][:])
                s["m_sb"] = esbp.tile([81, NG], f32r, tag="m", name=f"m{h}")
                nc.vector.tensor_mul(s["m_sb"][:], s["g_ps"][:], s["tb_sb"][:])

            def emit_A2(h):
                s = st[h]
                s["bil_ps"] = epp.tile([9, NG], f32, tag="ep", name=f"bil{h}")
                nc.tensor.matmul(s["bil_ps"][:], cs["rp"][:], s["m_sb"][:])
                s["bil_sb"] = esbp.tile([9, NG], f32, tag="bil",
                                        name=f"bils{h}")
                nc.scalar.copy(s["bil_sb"][:], s["bil_ps"][:])
                s["sq_sb"] = esbp.tile([9, NG], f32r, tag="sq", name=f"sq{h}")
                nc.vector.tensor_mul(s["sq_sb"][:], s["bil_sb"][:],
                                     s["bil_sb"][:])

            def emit_A3(h):
                s = st[h]
                s["var_ps"] = epp.tile([1, NG], f32, tag="ep", name=f"var{h}")
                nc.tensor.matmul(s["var_ps"][:], cs["o99"][:], s["sq_sb"][:])
                s["std_sb"] = esbp.tile([1, NG], f32r, tag="std",
                                        name=f"std{h}")
                nc.scalar.activation(s["std_sb"][:], s["var_ps"][:],
                                     mybir.ActivationFunctionType.Sqrt,
                                     bias=cs["eps"][:, 0:1])

            def emit_A4(h):
                s = st[h]
                s["rb_ps"] = epp.tile([9, NG], f32, tag="ep", name=f"rb{h}")
                nc.tensor.matmul(s["rb_ps"][:], cs["o19"][:], s["std_sb"][:])
                s["rstd_sb"] = esbp.tile([9, NG], f32, tag="rstd",
                                         name=f"rstd{h}")
                nc.vector.reciprocal_approx_fast(s["rstd_sb"][:],
                                                 s["rb_ps"][:])
                s["ln_sb"] = esbp.tile([9, NG], f32r, tag="ln", name=f"ln{h}")
                nc.vector.tensor_mul(s["ln_sb"][:], s["bil_sb"][:],
                                     s["rstd_sb"][:])

            def emit_B1(h):
                s = st[h]
                s["o_ps"] = epp.tile([9, NG], f32, tag="ep", name=f"o{h}")
                nc.tensor.matmul(s["o_ps"][:], cs["l2"][:], s["ln_sb"][:],
                                 start=True, stop=False)
                nc.tensor.matmul(s["o_ps"][:], cs["l1"][:],
                                 s["y_sb"][0:9, :], start=False, stop=True)
                s["osb"] = esbp.tile([9, NG], f32, tag="osb", name=f"osb{h}")
                nc.vector.tensor_scalar_add(s["osb"][:], s["o_ps"][:],
                                            cs["bout"][:, 0:1])

            def emit_B2(h):
                s = st[h]
                # chunk s covers columns s*128.. = rows 4r+s, so partition r
                # holds rows 4r..4r+3 (144B contiguous store runs)
                ot_ps = yp.tile([128, NCH, 9], f32, tag="y", name=f"ot{h}")
                for sx in range(NCH):
                    nc.tensor.matmul(
                        ot_ps[:, sx, :],
                        s["osb"][:, sx * 128:(sx + 1) * 128],
                        cs["i9"][:],
                        is_transpose=True,
                        start=(sx == 0), stop=(sx == NCH - 1))
                out_sb = outp.tile([128, NCH, 9], f32, tag="out_sb",
                                   name=f"outsb{h}")
                nc.vector.tensor_copy(out_sb[:], ot_ps[:])
                nc.scalar.dma_start(
                    out_d.ap()[h * NG:(h + 1) * NG, :]
                    .rearrange("(p s) k -> p s k", s=NCH),
                    out_sb[:])
                del st[h]

            # 3-deep software pipeline; PE emission order hand-interleaved
            for g in range(ngrp + 2):
                h1, h2 = g - 1, g - 2
                if g < ngrp:
                    emit_load(g)
                    emit_T(g, 0)
                    emit_T(g, 1)
                if 0 <= h1 < ngrp:
                    emit_A1(h1)
                if 0 <= h2 < ngrp:
                    emit_B1(h2)
                if g < ngrp:
                    emit_T(g, 2)
                if 0 <= h1 < ngrp:
                    emit_A2(h1)
                if g < ngrp:
                    emit_T(g, 3)
                if 0 <= h2 < ngrp:
                    emit_B2(h2)
                if g < ngrp:
                    emit_P(g, 0)
                if 0 <= h1 < ngrp:
                    emit_A3(h1)
                if g < ngrp:
                    emit_P(g, 1)
                if 0 <= h1 < ngrp:
                    emit_A4(h1)
                if g < ngrp:
                    emit_ycopy(g)
    nc.compile()
    return nc


_PROGRAM = None


def _get_program():
    global _PROGRAM
    if _PROGRAM is None:
        _PROGRAM = build_program()
    return _PROGRAM


def kernel(feat, W_int, W_stim, trans, ln_w, ln_b, W_out, b_out,
           trace=False, trace_kwargs=None):
    _ensure_axon_hooks()
    from concourse.bass_utils import run_bass_kernel_spmd

    feat = np.asarray(feat, np.float32)
    feat2 = feat.reshape(B_FULL, D)
    consts = make_consts(W_int, W_stim, trans, ln_w, ln_b, W_out, b_out)
    nc = _get_program()
    in_maps = []
    for c in range(N_CORES):
        m = {"feat": np.ascontiguousarray(feat2[c * B_CORE:(c + 1) * B_CORE])}
        m.update(consts)
        in_maps.append(m)
    res = run_bass_kernel_spmd(nc, in_maps, list(range(N_CORES)), trace=trace)
    out = np.concatenate([res.results[c]["out"] for c in range(N_CORES)], axis=0)
    kernel.last_results = res
    return np.ascontiguousarray(out, dtype=np.float32)
